# revision 38
# baseline (speedup 1.0000x reference)
"""Causal self-attention (RoPE quirk variant) on 8 Trainium2 NeuronCores.

Reference computation (B=2, T=1024, C=2048, H=64 heads, hd=32):
  qkv = x @ w_attn; split q,k,v; RoPE(dim=n_head quirk) on q,k;
  causal softmax attention; y @ w_proj.

Sharding: 8 cores = 2 batches x 4 head-quarter shards (16 heads / core).
Each core computes attention for its 16 heads on its batch and a partial
output projection (its 512 channels of the 2048-channel contraction);
the host sums the 4 partials per batch.

Device design (v2):
  * fp16 everywhere on the 16-bit path (x, w, q, k, v, es, y, out).
    Scores carry the 1/sqrt(hd) scale (folded into Wq host-side), so
    |S| <~ 5 and exp(S) fits fp16 comfortably.  PSUM/z stay fp32.
  * v is produced directly in [T, chan] layout (x-tile-stationary
    matmuls) -- no PE transposes.
  * scores per block land in ONE [128, 4, 512] psum tile (4 banks) so a
    single fused ACT exp covers all 4 heads of the block.
  * causal masking of diagonal blocks via a DVE multiply with a
    precomputed triangular fp16 mask (gpsimd affine_select dropped).
  * softmax denominators via the ones-matmul (col-packed with PV).
  * y normalization reads psum directly: recip(z) then psy * (1/z).
  * explicit software pipelining: attention blocks are interleaved with
    "filler" PE work (v projection, next group's q/k projection, the
    out-projection) so the PE never starves while ACT runs exp, and the
    HAM clock gate stays warm.
"""

import json
import os
import sys
from collections import deque
from itertools import chain

sys.path.insert(0, "/opt/trn_rl_repo")

import numpy as np

import concourse.bass as bass
import concourse.mybir as mybir
import concourse.tile as tile

F32 = mybir.dt.float32
F16 = mybir.dt.float16

INTERLEAVE = True

B, T, C = 2, 1024, 2048
H, HD = 64, 32
SCALE = 1.0 / np.sqrt(32.0)

_PATCHED = False


def _split_excess_waits(bir_json: bytes) -> bytes:
    """The walrus build in this container encodes at most ONE sync-wait per
    instruction; Tile's wait assigner emits several. Hoist excess waits onto
    same-engine NoOps placed immediately before the instruction."""
    d = json.loads(bir_json)
    ctr = 0
    for fn in d.get("functions", []):
        for blk in fn.get("blocks", []):
            out = []
            for inst in blk.get("instructions", []):
                si = inst.get("sync_info")
                waits = (si or {}).get("on_wait") or []
                if len(waits) > 1:
                    for w in waits[:-1]:
                        out.append({
                            "name": f"WSplit-{ctr}",
                            "opcode": "NoOp",
                            "engine": inst["engine"],
                            "ins": [],
                            "outs": [],
                            "sync_info": {"on_update": [], "on_wait": [w]},
                        })
                        ctr += 1
                    si["on_wait"] = [waits[-1]]
                out.append(inst)
            blk["instructions"] = out
    return json.dumps(d).encode()


def _install_patches():
    global _PATCHED
    if _PATCHED:
        return
    import concourse.bass_utils as bu
    import concourse.bass2jax as b2j

    orig = bu.compile_bir_kernel

    def patched_compile(bir_json, tmpdir, neff_name="file.neff"):
        return orig(_split_excess_waits(bir_json), tmpdir, neff_name)

    bu.compile_bir_kernel = patched_compile
    b2j.compile_bir_kernel = patched_compile
    _PATCHED = True


def _build_bass():
    nc = bass.Bass(trn_type="TRN2")
    xT = nc.dram_tensor("xT", [128, 16, 1024], F16, kind="ExternalInput").ap()
    wQK = nc.dram_tensor("wQK", [128, 8, 16, 128], F16, kind="ExternalInput").ap()
    wV = nc.dram_tensor("wV", [128, 16, 512], F16, kind="ExternalInput").ap()
    wP = nc.dram_tensor("wP", [128, 4, 2048], F16, kind="ExternalInput").ap()
    cosT = nc.dram_tensor("cosT", [128, 1024], F16, kind="ExternalInput").ap()
    sinT = nc.dram_tensor("sinT", [128, 1024], F16, kind="ExternalInput").ap()
    tri4 = nc.dram_tensor("tri4", [128, 4, 128], F16, kind="ExternalInput").ap()
    out = nc.dram_tensor("out", [1024, 2048], F16, kind="ExternalOutput").ap()
    outr = out.rearrange("(tq p) n -> tq p n", p=128)
    debug = os.environ.get("KBG_DEBUG") == "1"
    if debug:
        dbgq = nc.dram_tensor("dbgq", [128, 8, 1024], F16,
                              kind="ExternalOutput").ap()
        dbgv = nc.dram_tensor("dbgv", [128, 8, 512], F16,
                              kind="ExternalOutput").ap()
        dbgy = nc.dram_tensor("dbgy", [128, 4, 1024], F16,
                              kind="ExternalOutput").ap()
        dbge = nc.dram_tensor("dbge", [128, 4, 512], F16,
                              kind="ExternalOutput").ap()

    EXP = mybir.ActivationFunctionType.Exp

    with tile.TileContext(nc) as tc:
        with tc.tile_pool(name="persist", bufs=1) as persist, \
             tc.tile_pool(name="xpool", bufs=1) as xpool, \
             tc.tile_pool(name="wstream", bufs=3) as wsp, \
             tc.tile_pool(name="qtp", bufs=2) as qtp, \
             tc.tile_pool(name="esp", bufs=6) as esp, \
             tc.tile_pool(name="osb", bufs=4) as osb, \
             tc.tile_pool(name="zrp", bufs=2) as zrp, \
             tc.tile_pool(name="psA", bufs=1, space="PSUM") as psa, \
             tc.tile_pool(name="psS", bufs=2, space="PSUM") as psS, \
             tc.tile_pool(name="psYZ", bufs=1, space="PSUM") as psYZ:

            qkT = persist.tile([128, 8, 1024], F16)     # rotated q (0-3) / k (4-7)
            v_sb = persist.tile([128, 8, 512], F16)     # [T_k part, kb, chan]
            cos_sb = persist.tile([128, 1024], F16)
            sin_sb = persist.tile([128, 1024], F16)     # sign-folded
            ones_sb = persist.tile([128, 32], F16)
            tri_sb = persist.tile([128, 4, 128], F16)
            wv_sb = persist.tile([128, 16, 512], F16)
            wp_sb = persist.tile([128, 4, 2048], F16)
            warm = persist.tile([128, 16], F32)
            y_tiles = [persist.tile([128, 1024], F16, name=f"y{g}")
                       for g in range(4)]
            xt = xpool.tile([128, 16, 1024], F16)
            dbg_es = []

            # --- prelude: small tables + ACT exp table warm-up ---
            nc.scalar.dma_start(cos_sb, cosT)
            nc.scalar.dma_start(sin_sb, sinT)
            nc.scalar.dma_start(tri_sb, tri4)
            nc.vector.memset(ones_sb, 1.0)
            nc.vector.memset(warm, 0.0)
            nc.scalar.activation(warm, warm, EXP)

            # --- bulk input DMAs (sync ring): first group's weights, x, wv ---
            wt_first = [wsp.tile([128, 16, 128], F16, tag="wa", name=f"wt0_{j}")
                        for j in range(2)]
            nc.sync.dma_start(wt_first[0], wQK[:, 0])
            nc.sync.dma_start(wt_first[1], wQK[:, 4])
            for ko in range(16):
                nc.sync.dma_start(xt[:, ko, :], xT[:, ko, :])
            for c in range(4):
                nc.sync.dma_start(wv_sb[:, 4 * c:4 * c + 4, :],
                                  wV[:, 4 * c:4 * c + 4, :])

            # ---------- generators (each yield ~ one PE-instruction step) ----

            def gen_qk_dma(g):
                if g == 0:
                    yield
                    return
                for j, mi in enumerate((g, 4 + g)):
                    wt = wsp.tile([128, 16, 128], F16, tag="wa",
                                  name=f"wt{g}_{j}")
                    nc.sync.dma_start(wt, wQK[:, mi])
                    gen_qk_dma.cache[(g, j)] = wt
                    yield
            gen_qk_dma.cache = {}

            def gen_qk_mms(g):
                """q/k projection for group g into pre, then RoPE -> qkT."""
                pre = qtp.tile([128, 2, 1024], F16, tag="pre")
                swp = qtp.tile([128, 2, 1024], F16, tag="swp")
                for j in range(2):
                    if g == 0:
                        wt = wt_first[j]
                    else:
                        wt = gen_qk_dma.cache[(g, j)]
                    for half in range(2):
                        ps = psa.tile([128, 512], F32, tag=f"psA{half}",
                                      name=f"qk{g}_{j}_{half}")
                        c0 = half * 512
                        for ko in range(16):
                            nc.tensor.matmul(ps, wt[:, ko, :],
                                             xt[:, ko, c0:c0 + 512],
                                             start=ko == 0, stop=ko == 15)
                            yield
                        nc.vector.tensor_copy(pre[:, j, c0:c0 + 512], ps)
                        yield
                gen_qk_mms.pre[g] = (pre, swp)
            gen_qk_mms.pre = {}

            def gen_rope(g):
                pre, swp = gen_qk_mms.pre[g]
                pre_v = pre.rearrange("(a b) j f -> a b j f", b=2)
                swp_v = swp.rearrange("(a b) j f -> a b j f", b=2)
                nc.sync.dma_start(swp_v[:, 0], pre_v[:, 1])
                nc.sync.dma_start(swp_v[:, 1], pre_v[:, 0])
                yield
                for j, dst in enumerate((qkT[:, g, :], qkT[:, 4 + g, :])):
                    nc.gpsimd.tensor_mul(swp[:, j, :], swp[:, j, :], sin_sb)
                    yield
                    nc.vector.tensor_mul(pre[:, j, :], pre[:, j, :], cos_sb)
                    yield
                    nc.vector.tensor_add(dst, pre[:, j, :], swp[:, j, :])
                    yield

            def gen_v(tb_lo, tb_hi):
                """x-stationary projection: v in natural [T, chan] layout."""
                for tb in range(tb_lo, tb_hi):
                    psv = psa.tile([128, 512], F32, tag=f"psA{tb % 2}",
                                   name=f"v{tb}")
                    for ko in range(16):
                        nc.tensor.matmul(psv,
                                         xt[:, ko, tb * 128:(tb + 1) * 128],
                                         wv_sb[:, ko, :],
                                         start=ko == 0, stop=ko == 15)
                        yield
                    if tb < 2:
                        nc.scalar.copy(v_sb[:, tb, :], psv)
                    else:
                        nc.vector.tensor_copy(v_sb[:, tb, :], psv)
                    yield

            def gen_wp_dma():
                nc.scalar.dma_start(wp_sb, wP)
                yield

            def emit_pvz(g, kb, es, N, off, psy, psz, start, stop):
                for h in range(4):
                    c0 = g * 128 + 32 * h
                    nc.tensor.matmul(psy[32 * h:32 * h + 32, off:512],
                                     v_sb[:, kb, c0:c0 + 32],
                                     es[:, h, :N],
                                     start=start, stop=stop,
                                     tile_position=(0, 32 * h),
                                     skip_group_check=True)
                for h in range(4):
                    nc.tensor.matmul(psz[32 * h:32 * h + 32, off:512],
                                     ones_sb,
                                     es[:, h, :N],
                                     start=start, stop=stop,
                                     tile_position=(0, 32 * h),
                                     skip_group_check=True)

            def gen_att(g, qcs):
                """Attention blocks for group g.  Scores/exp run at head-pair
                granularity: each pair owns a full psum bank per head, and the
                2-bank pair tiles are double-buffered so the next block's
                score matmuls overlap the current block's ACT exp."""
                y_g = y_tiles[g]
                for qc in qcs:
                    q0 = qc * 512
                    nkb = (qc + 1) * 4
                    psy = psYZ.tile([128, 512], F32, tag="psy",
                                    name=f"psy{g}_{qc}")
                    psz = psYZ.tile([128, 512], F32, tag="psz",
                                    name=f"psz{g}_{qc}")
                    prev = None
                    for kb in range(nkb):
                        k0 = kb * 128
                        n0 = max(q0, k0)
                        N = q0 + 512 - n0
                        off = n0 - q0
                        es = esp.tile([128, 4, 512], F16, tag="es")
                        pss2 = []
                        for hp in range(2):
                            pss = psS.tile([128, 2, 512], F32, tag="pss")
                            pss2.append(pss)
                            for j in range(2):
                                h = 2 * hp + j
                                nc.tensor.matmul(
                                    pss[:, j, :N],
                                    qkT[32 * h:32 * h + 32, 4 + g,
                                        k0:k0 + 128],
                                    qkT[32 * h:32 * h + 32, g, n0:n0 + N],
                                    start=True, stop=True,
                                    tile_position=(32 * h, 0))
                        for hp in range(2):
                            nc.scalar.activation(
                                es[:, 2 * hp:2 * hp + 2, :N],
                                pss2[hp][:, :, :N], EXP)
                        if prev is not None:
                            emit_pvz(*prev)
                        if k0 >= q0:
                            nc.vector.tensor_mul(es[:, :, 0:128],
                                                 es[:, :, 0:128], tri_sb)
                        if debug and g == 0 and qc == 0 and kb == 0:
                            nc.sync.dma_start(dbge, es)
                        prev = (g, kb, es, N, off, psy, psz, kb == 0,
                                kb == nkb - 1)
                        yield
                    emit_pvz(*prev)
                    # y normalization straight from psum
                    zr = zrp.tile([128, 512], F32, tag="zr",
                                  name=f"zr{g}_{qc}")
                    nc.vector.reciprocal(zr, psz)
                    nc.vector.tensor_mul(y_g[:, q0:q0 + 512], psy, zr)
                    yield

            def gen_outproj(tq_list, cast_engine):
                for tq in tq_list:
                    for p in range(2):
                        pso = [psa.tile([128, 512], F32, tag=f"psA{n}",
                                        name=f"pso{tq}_{p}_{n}")
                               for n in range(2)]
                        for gk in range(4):
                            lhs = y_tiles[gk][:, tq * 128:(tq + 1) * 128]
                            for n in range(2):
                                nc.tensor.matmul(
                                    pso[n], lhs,
                                    wp_sb[:, gk, (2 * p + n) * 512:
                                          (2 * p + n + 1) * 512],
                                    start=gk == 0, stop=gk == 3)
                                yield
                        for n in range(2):
                            o_sb = osb.tile([128, 512], F16, tag="osb")
                            if cast_engine == "vector":
                                nc.vector.tensor_copy(o_sb, pso[n])
                            else:
                                nc.scalar.copy(o_sb, pso[n])
                            nc.scalar.dma_start(
                                outr[tq][:, (2 * p + n) * 512:
                                         (2 * p + n + 1) * 512], o_sb)
                            yield

            # ---------- driver: interleave attention with filler PE work ----

            def run(gen):
                for _ in gen:
                    pass

            def co_run(att_gen, filler, per_block):
                """One attention block, then `per_block` filler steps."""
                if not INTERLEAVE:
                    run(filler)
                    run(att_gen)
                    return
                for _ in att_gen:
                    for _ in range(per_block):
                        if next(filler, StopIteration) is StopIteration:
                            break
                run(filler)

            # seg A: q/k proj of group 0 + RoPE0 + first two v T-blocks
            run(gen_qk_mms(0))
            run(gen_rope(0))
            run(gen_v(0, 2))

            # seg B: att0 interleaved with [qk1 dma, v(2..8), qk1, rope1].
            # per_block=13 keeps each v T-block's cast emitted before the
            # attention PV that consumes it (deadlock audit in notes).
            filler_b = chain(gen_qk_dma(1), gen_v(2, 8), gen_qk_mms(1),
                             gen_rope(1))
            co_run(gen_att(0, (0, 1)), filler_b, 13)

            # seg C: att1 with [qk2 dma, wp dma, qk2, rope2]
            filler_c = chain(gen_qk_dma(2), gen_wp_dma(), gen_qk_mms(2),
                             gen_rope(2))
            co_run(gen_att(1, (0, 1)), filler_c, 6)

            # seg D: att2 with [qk3 dma, qk3, rope3]
            filler_d = chain(gen_qk_dma(3), gen_qk_mms(3), gen_rope(3))
            co_run(gen_att(2, (0, 1)), filler_d, 6)

            # seg E: att3 qc0 unaccompanied (short), then att3 qc1
            # interleaved with the first half of the out-projection.
            run(gen_att(3, (0,)))
            co_run(gen_att(3, (1,)), gen_outproj((0, 1, 2, 3), "vector"), 9)

            # seg F: remaining out-projection
            run(gen_outproj((4, 5, 6, 7), "scalar"))

            if debug:
                nc.sync.dma_start(dbgq, qkT)
                nc.sync.dma_start(dbgv, v_sb)
                for g in range(4):
                    nc.sync.dma_start(dbgy[:, g], y_tiles[g])
    return nc


_NC_CACHE = None


def _host_inputs(x, pos, w_attn, w_proj):
    """Build the 8 per-core input dicts."""
    x = np.asarray(x, dtype=np.float32)
    pos = np.asarray(pos, dtype=np.float32)
    w_attn = np.asarray(w_attn, dtype=np.float32)
    w_proj = np.asarray(w_proj, dtype=np.float32)

    TRI = (np.arange(128)[:, None] <= np.arange(128)[None, :]).astype(
        np.float16)
    tri4 = np.ascontiguousarray(np.tile(TRI[:, None, :], (1, 4, 1)))
    inv_freq = (1.0 / (10000.0 ** (np.arange(0, H, 2, dtype=np.float32) / H)))
    sinus = pos[:, None] * inv_freq[None, :]              # [T, 32]
    cosT = np.tile(np.cos(sinus).T, (4, 1))               # [128, T]
    sinT = np.tile(np.sin(sinus).T, (4, 1)).copy()
    sinT[0::2, :] *= -1.0                                 # rotate_half signs
    cosT = cosT.astype(np.float16)
    sinT = sinT.astype(np.float16)

    in_maps = []
    for core in range(8):
        b, gq = divmod(core, 4)
        hs = slice(gq * 512, (gq + 1) * 512)
        Wq = (w_attn[:, 0:2048][:, hs] * SCALE).astype(np.float32)
        Wk = w_attn[:, 2048:4096][:, hs]
        Wv = w_attn[:, 4096:6144][:, hs]
        WQK = np.concatenate([Wq, Wk], axis=1)            # [2048, 1024]
        wqk = np.ascontiguousarray(
            WQK.reshape(16, 128, 8, 128).transpose(1, 2, 0, 3)).astype(
            np.float16)                                   # ki mi ko mc
        wv = np.ascontiguousarray(
            Wv.reshape(16, 128, 512).transpose(1, 0, 2)).astype(np.float16)
        wPr = np.ascontiguousarray(
            w_proj[hs, :].reshape(4, 128, 2048).transpose(1, 0, 2)).astype(
            np.float16)
        xTr = np.ascontiguousarray(
            x[b].T.reshape(16, 128, 1024).transpose(1, 0, 2)).astype(
            np.float16)
        in_maps.append({
            "xT": xTr, "wQK": wqk, "wV": wv, "wP": wPr,
            "cosT": cosT, "sinT": sinT, "tri4": tri4,
        })
    return in_maps


def kernel(x, pos, w_attn, w_proj, _trace=False):
    global _NC_CACHE
    _install_patches()
    from concourse.bass_utils import run_bass_kernel_spmd

    if _NC_CACHE is None:
        _NC_CACHE = _build_bass()
    nc = _NC_CACHE
    in_maps = _host_inputs(x, pos, w_attn, w_proj)
    res = run_bass_kernel_spmd(nc, in_maps, core_ids=list(range(8)), trace=_trace)
    outs = [np.asarray(res.results[c]["out"], dtype=np.float32)
            for c in range(8)]
    full = np.stack([
        outs[0] + outs[1] + outs[2] + outs[3],
        outs[4] + outs[5] + outs[6] + outs[7],
    ]).astype(np.float32)
    kernel.last_results = res
    return full
